# revision 1
# baseline (speedup 1.0000x reference)
"""LatticeLSTM (BiLSTM w/ word cells) Trainium2 kernel.

Sharding: time-sharded across 8 cores. Core k computes local window
[st(k), st(k)+96) of the 512-step scan for ALL 64 lanes (32 batch fw +
32 batch bw), where st(0)=0 and st(k)=64k-32 for k>0. The 32-step
warm-up from zero state converges to the true state (coupled forget
gate contracts ~0.5/step; validated max abs err 5e-7 at W=32), so each
core's last 64 steps (first 64 for core 0) are exact. No collectives.

Device layout: "layout B" — feature/gate index on SBUF partitions,
lanes on the free dim. Recurrent matmuls are weight-stationary:
out[gates, lanes] = W_tile^T @ h^T. fw/bw lanes use separate weight
sets, so each matmul covers one direction (N=32).

Per-step PSUM banks:
  pg [128, 8, 64]: pre-gates i(2) o(2) g(2) + alpha(2), chunk-major
  pw [128, 6, 64]: word gates iw(2) fw(2) gw(2)
Biases are injected by a [K=chunks, M=128] x [K, chunks*lanes] selector
matmul (start=True clears the bank), then x-projections and h-matmuls
accumulate on top.

Masks (merge / has-word) depend only on the integer word-lattice inputs
and are precomputed on host, as is the skip_input reversal; embedding
rows are gathered+transposed on-device via dma_gather (bf16).
"""

import numpy as np
import ml_dtypes

import concourse.bass as bass
import concourse.bacc as bacc
import concourse.tile as tile
from concourse import mybir
from concourse.bass_utils import run_bass_kernel_spmd

B, S, E, H, V, L = 32, 512, 128, 256, 21128, 32
NCORES = 8
WARM = 32
CHUNK = 64
T = CHUNK + WARM            # 96 local steps per core
LANES = 64                  # 32 fw + 32 bw
NIDX = T * LANES            # 6144 gathered rows per table
NT = 12                     # tag matmul: steps per N-chunk (8 chunks of 12)

f32 = mybir.dt.float32
bf16 = mybir.dt.bfloat16
i16 = mybir.dt.int16
i8 = mybir.dt.int8
Sig = mybir.ActivationFunctionType.Sigmoid
Tanh = mybir.ActivationFunctionType.Tanh

bf = ml_dtypes.bfloat16

_CACHE = {}


def _chunk_bcast(ap2, nchunk=2):
    """[128, 64] AP -> [128, nchunk, 64] with zero-stride chunk dim."""
    return bass.AP(tensor=ap2.tensor, offset=ap2.offset,
                   ap=[ap2.ap[0], [0, nchunk], ap2.ap[1]])


def _build_bass():
    nc = bacc.Bacc(None, target_bir_lowering=False)

    def inp(name, shape, dtype):
        return nc.declare_dram_parameter(name, list(shape), dtype, isOutput=False)

    xT_d = inp("x_T", [128, NIDX], bf16)
    weT_d = inp("we_T", [128, NIDX], bf16)
    # x-side weight tiles (lhsT): [K=E rows, m chunks, 128 gate cols]
    wih_d = {d: inp(f"wih_{d}", [E, 6, 128], bf16) for d in "fb"}
    awih_d = {d: inp(f"awih_{d}", [E, 2, 128], bf16) for d in "fb"}
    wwih_d = {d: inp(f"wwih_{d}", [E, 6, 128], bf16) for d in "fb"}
    # h-side weight tiles: [128 K-rows, kc, m, 128]
    whh_d = {d: inp(f"whh_{d}", [128, 2, 6, 128], bf16) for d in "fb"}
    wwhh_d = {d: inp(f"wwhh_{d}", [128, 2, 6, 128], bf16) for d in "fb"}
    awhh_d = {d: inp(f"awhh_{d}", [128, 2, 2, 128], bf16) for d in "fb"}
    # bias lhsT rows: pg bank [8,128] (b chunks 0..5, ab chunks 6,7); pw bank [6,128]
    bg_d = {d: inp(f"biasg_{d}", [8, 128], bf16) for d in "fb"}
    bw_d = {d: inp(f"biasw_{d}", [6, 128], bf16) for d in "fb"}
    selg_d = inp("selg", [8, 8 * 32], bf16)   # sel[k, c*32+l] = (c==k)
    selw_d = inp("selw", [6, 6 * 32], bf16)
    maskm_d = inp("mask_m", [T, LANES], f32)
    maskw_d = inp("mask_w", [T, LANES], f32)
    tagw_d = inp("tagw", [128, 2, 2, 32], bf16)   # [K-row, dir, kc, label]

    out_d = nc.declare_dram_parameter("out_tags", [2, 32, T * 32], f32, isOutput=True)

    with tile.TileContext(nc) as tc:
        with (
            tc.tile_pool(name="const", bufs=1) as cpool,
            tc.tile_pool(name="state", bufs=1) as spool,
            tc.tile_pool(name="work", bufs=3) as wpool,
            tc.tile_pool(name="outp", bufs=4) as opool,
            tc.tile_pool(name="psumG", bufs=2, space="PSUM") as psG,
            tc.tile_pool(name="psumW", bufs=2, space="PSUM") as psW,
            tc.tile_pool(name="psumT", bufs=2, space="PSUM") as psT,
        ):
            # ---- load constants ----
            def load(dram, shape, dtype, tag):
                t_ = cpool.tile(list(shape), dtype, tag=tag)
                nc.sync.dma_start(out=t_[...], in_=dram[...])
                return t_

            wih = {d: load(wih_d[d], [E, 6, 128], bf16, f"wih{d}") for d in "fb"}
            awih = {d: load(awih_d[d], [E, 2, 128], bf16, f"awih{d}") for d in "fb"}
            wwih = {d: load(wwih_d[d], [E, 6, 128], bf16, f"wwih{d}") for d in "fb"}
            whh = {d: load(whh_d[d], [128, 2, 6, 128], bf16, f"whh{d}") for d in "fb"}
            wwhh = {d: load(wwhh_d[d], [128, 2, 6, 128], bf16, f"wwhh{d}") for d in "fb"}
            awhh = {d: load(awhh_d[d], [128, 2, 2, 128], bf16, f"awhh{d}") for d in "fb"}
            bg = {d: load(bg_d[d], [8, 128], bf16, f"bg{d}") for d in "fb"}
            bw_ = {d: load(bw_d[d], [6, 128], bf16, f"bw{d}") for d in "fb"}
            selg = load(selg_d, [8, 256], bf16, "selg")
            selw = load(selw_d, [6, 192], bf16, "selw")
            tagw = load(tagw_d, [128, 2, 2, 32], bf16, "tagw")

            # masks broadcast to all 128 partitions
            maskm = cpool.tile([128, T, LANES], f32, tag="maskm")
            maskw = cpool.tile([128, T, LANES], f32, tag="maskw")
            for md, mt in ((maskm_d, maskm), (maskw_d, maskw)):
                src = md[...]
                bsrc = bass.AP(tensor=src.tensor, offset=src.offset,
                               ap=[[0, 128]] + list(src.ap))
                nc.sync.dma_start(out=mt[...], in_=bsrc)

            # absorb the mask-DMA completion wait on DVE's vector clock here:
            # copy_predicated (3-AP ISA struct) has only ONE sync-wait slot.
            mwarm = cpool.tile([128, LANES], f32, tag="mwarm")
            nc.vector.tensor_copy(mwarm[...], maskm[:, 0, :])
            nc.vector.tensor_copy(mwarm[...], maskw[:, 0, :])

            # ---- embedding columns (host-gathered, transposed) ----
            x_T = load(xT_d, [128, NIDX], bf16, "xT")
            we_T = load(weT_d, [128, NIDX], bf16, "weT")

            # ---- states ----
            h_hist = spool.tile([128, T + 1, 2, 64], bf16)
            c_st = spool.tile([128, 2, 64], f32)
            pc_st = spool.tile([128, 2, 64], f32)
            pc_bf = spool.tile([128, 2, 64], bf16)
            nc.vector.memset(h_hist[:, 0, :, :], 0.0)
            nc.vector.memset(c_st[...], 0.0)
            nc.vector.memset(pc_st[...], 0.0)
            nc.vector.memset(pc_bf[...], 0.0)

            DIRS = (("f", 0), ("b", 32))

            def xcol(tile_, t, l0, n=32):
                return tile_[:, t * LANES + l0: t * LANES + l0 + n]

            def emit_pg_inject(pg, t):
                """bias + x-side products for step t into pg (opens group)."""
                first = True
                for d, l0 in DIRS:
                    nc.tensor.matmul(pg[:, :, l0:l0 + 32], bg[d][...], selg[...],
                                     start=first, stop=False)
                    first = False
                for d, l0 in DIRS:
                    for m in range(6):
                        nc.tensor.matmul(pg[:, m:m + 1, l0:l0 + 32],
                                         wih[d][:, m, :], xcol(x_T, t, l0),
                                         start=False, stop=False)
                    for m in range(2):
                        nc.tensor.matmul(pg[:, 6 + m:7 + m, l0:l0 + 32],
                                         awih[d][:, m, :], xcol(x_T, t, l0),
                                         start=False, stop=False)

            def emit_pg_h(pg, t):
                """pre-h into pg for step t (reads h_{t-1} = slot t)."""
                for d, l0 in DIRS:
                    for kc in range(2):
                        for m in range(6):
                            nc.tensor.matmul(pg[:, m:m + 1, l0:l0 + 32],
                                             whh[d][:, kc, m, :],
                                             h_hist[:, t, kc, l0:l0 + 32],
                                             start=False, stop=False)

            def emit_pg_alpha(pg):
                """alpha-h (pc) into pg; closes the group."""
                n = 0
                for d, l0 in DIRS:
                    for kc in range(2):
                        for m in range(2):
                            n += 1
                            nc.tensor.matmul(pg[:, 6 + m:7 + m, l0:l0 + 32],
                                             awhh[d][:, kc, m, :],
                                             pc_bf[:, kc, l0:l0 + 32],
                                             start=False, stop=(n == 8))

            def emit_pw(pw, t):
                first = True
                for d, l0 in DIRS:
                    nc.tensor.matmul(pw[:, :, l0:l0 + 32], bw_[d][...], selw[...],
                                     start=first, stop=False)
                    first = False
                for d, l0 in DIRS:
                    for m in range(6):
                        nc.tensor.matmul(pw[:, m:m + 1, l0:l0 + 32],
                                         wwih[d][:, m, :], xcol(we_T, t, l0),
                                         start=False, stop=False)

            def emit_pw_h(pw, t):
                n = 0
                for d, l0 in DIRS:
                    for kc in range(2):
                        for m in range(6):
                            n += 1
                            nc.tensor.matmul(pw[:, m:m + 1, l0:l0 + 32],
                                             wwhh[d][:, kc, m, :],
                                             h_hist[:, t + 1, kc, l0:l0 + 32],
                                             start=False, stop=(n == 24))

            # prologue: pg for step 0 (h_{-1}=0, pc=0 tiles)
            pg = psG.tile([128, 8, 64], f32)
            emit_pg_inject(pg, 0)
            emit_pg_h(pg, 0)
            emit_pg_alpha(pg)

            for t in range(T):
                m2 = _chunk_bcast(maskm[:, t, :])
                w2 = _chunk_bcast(maskw[:, t, :])

                # ---- V1: gates -> c_new, h_t ----
                sio = wpool.tile([128, 4, 64], f32)
                gt = wpool.tile([128, 2, 64], f32)
                al = wpool.tile([128, 2, 64], f32)
                nc.scalar.activation(sio[...], pg[:, 0:4, :], Sig)
                nc.scalar.activation(gt[...], pg[:, 4:6, :], Tanh)
                nc.scalar.activation(al[...], pg[:, 6:8, :], Sig)
                # q = c + m*(pc-c); e = i + m*(s-i)  (m is a 0/1 f32 mask)
                qd = wpool.tile([128, 2, 64], f32)
                qm = wpool.tile([128, 2, 64], f32)
                q = wpool.tile([128, 2, 64], f32)
                nc.gpsimd.tensor_sub(qd[...], pc_st[...], c_st[...])
                nc.gpsimd.tensor_mul(qm[...], qd[...], m2)
                nc.gpsimd.tensor_add(q[...], c_st[...], qm[...])
                spre = wpool.tile([128, 2, 64], f32)
                nc.gpsimd.tensor_sub(spre[...], sio[:, 0:2, :], al[...])
                s = wpool.tile([128, 2, 64], f32)
                nc.scalar.activation(s[...], spre[...], Sig)
                se = wpool.tile([128, 2, 64], f32)
                sm = wpool.tile([128, 2, 64], f32)
                e = wpool.tile([128, 2, 64], f32)
                nc.vector.tensor_sub(se[...], s[...], sio[:, 0:2, :])
                nc.vector.tensor_mul(sm[...], se[...], m2)
                nc.vector.tensor_add(e[...], sio[:, 0:2, :], sm[...])
                d_ = wpool.tile([128, 2, 64], f32)
                nc.gpsimd.tensor_sub(d_[...], gt[...], q[...])
                ed = wpool.tile([128, 2, 64], f32)
                nc.vector.tensor_mul(ed[...], e[...], d_[...])
                nc.vector.tensor_add(c_st[...], q[...], ed[...])
                tc_ = wpool.tile([128, 2, 64], f32)
                nc.scalar.activation(tc_[...], c_st[...], Tanh)
                nc.vector.tensor_mul(h_hist[:, t + 1, :, :], sio[:, 2:4, :], tc_[...])

                # ---- word-cell matmuls (and next-step injects) ----
                pw = psW.tile([128, 6, 64], f32)
                emit_pw(pw, t)
                if t < T - 1:
                    pg2 = psG.tile([128, 8, 64], f32)
                    emit_pg_inject(pg2, t + 1)
                emit_pw_h(pw, t)

                # ---- V2: word cell -> pc ----
                siofw = wpool.tile([128, 4, 64], f32)
                tgw = wpool.tile([128, 2, 64], f32)
                nc.scalar.activation(siofw[...], pw[:, 0:4, :], Sig)
                nc.scalar.activation(tgw[...], pw[:, 4:6, :], Tanh)
                t1 = wpool.tile([128, 2, 64], f32)
                t2 = wpool.tile([128, 2, 64], f32)
                nc.vector.tensor_mul(t1[...], siofw[:, 2:4, :], c_st[...])
                nc.gpsimd.tensor_mul(t2[...], siofw[:, 0:2, :], tgw[...])
                cw = wpool.tile([128, 2, 64], f32)
                nc.vector.tensor_add(cw[...], t1[...], t2[...])
                # pc += w*(cw-pc)
                pd = wpool.tile([128, 2, 64], f32)
                pm = wpool.tile([128, 2, 64], f32)
                nc.vector.tensor_sub(pd[...], cw[...], pc_st[...])
                nc.vector.tensor_mul(pm[...], pd[...], w2)
                nc.vector.tensor_add(pc_st[...], pc_st[...], pm[...])
                nc.vector.tensor_copy(pc_bf[...], pc_st[...])

                if t < T - 1:
                    emit_pg_h(pg2, t + 1)
                    emit_pg_alpha(pg2)
                    pg = pg2

            # ---- tag projection: out[d, label, tau*32+lane] ----
            for di, (d, l0) in enumerate(DIRS):
                for n in range(T // NT):
                    pt = psT.tile([32, NT, 32], f32)
                    for kc in range(2):
                        nc.tensor.matmul(
                            pt[...], tagw[:, di, kc, :],
                            h_hist[:, 1 + n * NT: 1 + (n + 1) * NT, kc, l0:l0 + 32],
                            start=(kc == 0), stop=(kc == 1))
                    ob = opool.tile([32, NT, 32], f32)
                    nc.vector.tensor_copy(ob[...], pt[...])
                    nc.sync.dma_start(
                        out=out_d[di, :, n * NT * 32: (n + 1) * NT * 32],
                        in_=ob[...])

    nc.compile()
    return nc


# ------------------------- host side -------------------------

def _window_start(k):
    return 0 if k == 0 else 64 * k - WARM


def _masks_for_window(wlen_win):
    """wlen_win [32, T] int -> merge mask m [T,32], has-word hw [T,32] f32,
    replicating the truncated-from-zero pcnt/pvalid recurrence."""
    n = wlen_win.shape[0]
    pcnt = np.full((n,), -1, np.int64)
    pvalid = np.zeros((n,), bool)
    m = np.zeros((T, n), np.float32)
    hw = np.zeros((T, n), np.float32)
    for t in range(T):
        mg = pvalid & (pcnt == 0)
        m[t] = mg.astype(np.float32)
        pvalid = pvalid & ~mg
        pcnt = pcnt - 1
        w = wlen_win[:, t] >= 2
        hw[t] = w.astype(np.float32)
        pcnt = np.where(w, wlen_win[:, t] - 1, pcnt)
        pvalid = pvalid | w
    return m, hw


def _wrap_idx(flat):
    """[NIDX] -> [128, NIDX//16] int16 (idx i at [i%16, i//16])."""
    out = np.zeros((128, NIDX // 16), np.int16)
    out[:16] = flat.reshape(-1, 16).T
    return out


def _weight_tiles(Wx, Whh, aWx, aWhh, wWx, wWhh, b, ab, wb):
    r = {}
    r["wih"] = np.ascontiguousarray(Wx.reshape(E, 6, 128)).astype(bf)
    r["awih"] = np.ascontiguousarray(aWx.reshape(E, 2, 128)).astype(bf)
    r["wwih"] = np.ascontiguousarray(wWx.reshape(E, 6, 128)).astype(bf)
    r["whh"] = np.ascontiguousarray(
        Whh.reshape(2, 128, 6, 128).transpose(1, 0, 2, 3)).astype(bf)
    r["wwhh"] = np.ascontiguousarray(
        wWhh.reshape(2, 128, 6, 128).transpose(1, 0, 2, 3)).astype(bf)
    r["awhh"] = np.ascontiguousarray(
        aWhh.reshape(2, 128, 2, 128).transpose(1, 0, 2, 3)).astype(bf)
    bgv = np.zeros((8, 128), np.float32)
    bgv[:6] = b.reshape(6, 128)
    bgv[6:] = ab.reshape(2, 128)
    r["biasg"] = bgv.astype(bf)
    r["biasw"] = wb.reshape(6, 128).astype(bf)
    return r


def _prep(inputs):
    inputs = {k: np.asarray(v) for k, v in inputs.items()}
    cids = inputs["component_ids"].astype(np.int64)
    skip = inputs["skip_input"].astype(np.int64)
    wid, wlen = skip[..., 0], skip[..., 1]

    # reference's skip reversal
    tt = np.arange(S)[None, :]
    valid = wlen > 0
    rev_pos = np.where(valid, S - tt - wlen, S)
    skip_rev = np.zeros((B, S + 1, 2), np.int64)
    bidx = np.broadcast_to(np.arange(B)[:, None], (B, S))
    skip_rev[bidx, rev_pos] = skip * valid[..., None]
    skip_rev = skip_rev[:, :S]
    cids_r = cids[:, ::-1]
    wid_r, wlen_r = skip_rev[..., 0], skip_rev[..., 1]

    emb_bf = inputs["emb"].astype(bf)
    emb_bf32 = emb_bf.astype(np.float32)   # for exact-cast transposed gathers

    wt = {}
    for d, pre in (("f", "fw_"), ("b", "bw_")):
        a = [inputs[pre + n] for n in
             ["Wih", "Whh", "aWih", "aWhh", "wWih", "wWhh", "b", "ab", "wb"]]
        wt[d] = _weight_tiles(*a)

    selg = np.zeros((8, 256), np.float32)
    for c in range(8):
        selg[c, c * 32:(c + 1) * 32] = 1.0
    selw = np.zeros((6, 192), np.float32)
    for c in range(6):
        selw[c, c * 32:(c + 1) * 32] = 1.0

    tag = np.zeros((128, 2, 2, 32), np.float32)
    tw = inputs["tag_W"]          # [512, 32]
    for di in range(2):
        for kc in range(2):
            tag[:, di, kc, :] = tw[256 * di + 128 * kc: 256 * di + 128 * (kc + 1), :]

    shared = {"selg": selg.astype(bf), "selw": selw.astype(bf),
              "tagw": tag.astype(bf)}
    for d in "fb":
        for nm in ["wih", "awih", "wwih", "whh", "wwhh", "awhh", "biasg", "biasw"]:
            shared[f"{nm}_{d}"] = wt[d][nm]

    in_maps = []
    for k in range(NCORES):
        st = _window_start(k)
        xf = cids[:, st:st + T]          # [32, T]
        xb = cids_r[:, st:st + T]
        wf = wid[:, st:st + T]
        wb2 = wid_r[:, st:st + T]
        xflat = np.concatenate([xf.T, xb.T], axis=1).reshape(-1)   # [T*64]
        wflat = np.concatenate([wf.T, wb2.T], axis=1).reshape(-1)
        mf, hf = _masks_for_window(wlen[:, st:st + T])
        mb, hb = _masks_for_window(wlen_r[:, st:st + T])
        mask_m = np.concatenate([mf, mb], axis=1).astype(np.float32)   # [T, 64]
        mask_w = np.concatenate([hf, hb], axis=1).astype(np.float32)
        im = dict(shared)
        im["x_T"] = np.ascontiguousarray(emb_bf32[xflat, :].T).astype(bf)
        im["we_T"] = np.ascontiguousarray(emb_bf32[wflat, :].T).astype(bf)
        im["mask_m"] = mask_m
        im["mask_w"] = mask_w
        in_maps.append(im)
    return in_maps


def _postprocess(results, inputs):
    tag_b = np.asarray(inputs["tag_b"])
    out = np.zeros((B, S, L), np.float32)
    for k in range(NCORES):
        st = _window_start(k)
        t0 = 0 if k == 0 else WARM
        arr = results[k]["out_tags"]              # [2, 32, T*32]
        fwp = arr[0].reshape(L, T, 32).transpose(2, 1, 0)   # [batch, tau, L]
        bwp = arr[1].reshape(L, T, 32).transpose(2, 1, 0)
        gsl = np.arange(64) + st + t0
        out[:, gsl, :] += fwp[:, t0:t0 + 64, :]
        out[:, S - 1 - gsl, :] += bwp[:, t0:t0 + 64, :]
    return out + tag_b[None, None, :]


def _ensure_ntff_hook():
    """The image's antenv lacks axon_hooks; shim it so trace=True works."""
    import sys
    import types
    try:
        from antenv.axon_hooks import get_axon_ntff_profile_hook  # noqa: F401
        return
    except ImportError:
        pass
    import antenv
    from trn_agent_boot.trn_boot import _ntff_profile_via_ctypes
    mod = types.ModuleType("antenv.axon_hooks")
    _state = {"h": _ntff_profile_via_ctypes("/opt/axon/libaxon_pjrt.so")}
    mod.set_axon_ntff_profile_hook = lambda h: _state.__setitem__("h", h)
    mod.get_axon_ntff_profile_hook = lambda: _state["h"]
    sys.modules["antenv.axon_hooks"] = mod
    antenv.axon_hooks = mod


def run(inputs, trace=False):
    if trace:
        _ensure_ntff_hook()
    if "nc" not in _CACHE:
        _CACHE["nc"] = _build_bass()
    nc = _CACHE["nc"]
    in_maps = _prep(inputs)
    res = run_bass_kernel_spmd(nc, in_maps, core_ids=list(range(NCORES)),
                               trace=trace)
    out = _postprocess(res.results, {k: np.asarray(v) for k, v in inputs.items()})
    return out, res


def kernel(**inputs):
    out, _ = run(inputs, trace=False)
    return out



# revision 6
# speedup vs baseline: 2.0269x; 2.0269x over previous
"""LatticeLSTM (BiLSTM w/ word cells) Trainium2 kernel, v2.

Sharding: time-sharded, 2 windows per core. The 512-step scan splits
into 16 windows of C=32 owned steps; core k runs windows {2k, 2k+1}
simultaneously as extra lanes. Each window computes T = W + C = 48
local steps, where the first W=16 steps warm up the state from zero
(coupled forget gate contracts ~0.5/step, so warm-start error is
~2^-14 by the first owned step). Window 0 starts at global step 0
with no warm-up (its last W steps are wasted instead). No
collectives.

Device layout: feature/gate index on SBUF partitions, lanes on the
free dim. Lanes per direction L = 64 (2 windows x 32 batch). The fw
and bw recurrences are fully independent streams with separate PSUM
banks, weights, and work tiles, emitted interleaved so their serial
elementwise chains pipeline across the Act/DVE/GpSimd engines.

Per-step PSUM banks (per direction):
  pg [128, 8, 64]: pre-gates, chunk order i(2) o(2) alpha(2) g(2) so
     one Sigmoid covers chunks 0:6 and one Tanh covers 6:8.
  pw [128, 6, 64]: word gates iw(2) fw(2) gw(2).
Biases are injected by a [K=chunks, M=128] x [K, chunks*lanes]
selector matmul (start=True clears the bank), then x-projections and
h-matmuls accumulate on top.

The three mask-lerps (merge cell, merge input gate, pending-word
update) use in-place copy_predicated instead of 3-op lerps. Masks
depend only on the integer word-lattice inputs and are precomputed on
host, as is the skip_input reversal; embedding rows are gathered on
host (bf16).
"""

import numpy as np
import ml_dtypes

import concourse.bass as bass
import concourse.bacc as bacc
import concourse.tile as tile
from concourse import mybir
from concourse.bass_utils import run_bass_kernel_spmd

B, S, E, H, V, L = 32, 512, 128, 256, 21128, 32
NCORES = 8
G = 2                       # windows per core
C = 32                      # owned steps per window
WARM = 16
T = C + WARM                # 48 local steps
LD = 32 * G                 # lanes per direction (64)
NIDX = T * LD               # gathered rows per table per direction
NT = 8                      # tag matmul: steps per N-chunk

f32 = mybir.dt.float32
bf16 = mybir.dt.bfloat16
u8 = mybir.dt.uint8
Sig = mybir.ActivationFunctionType.Sigmoid
Tanh = mybir.ActivationFunctionType.Tanh

bf = ml_dtypes.bfloat16

_CACHE = {}

DIRS = ("f", "b")
# whh tile chunk mi -> pg chunk (i,i,o,o,g,g -> 0,1,2,3,6,7)
PG_HCH = (0, 1, 2, 3, 6, 7)


def _flat2(ap3):
    """[128, a, b] AP with contiguous free dims -> [128, a*b]."""
    p, (sa, ca), (sb, cb) = ap3.ap
    assert sa == sb * cb, f"non-contiguous free dims: {ap3.ap}"
    return bass.AP(tensor=ap3.tensor, offset=ap3.offset,
                   ap=[p, [sb, ca * cb]])


def _build_bass():
    nc = bacc.Bacc(None, target_bir_lowering=False)

    def inp(name, shape, dtype):
        return nc.declare_dram_parameter(name, list(shape), dtype, isOutput=False)

    xT_d = {d: inp(f"x_T_{d}", [128, NIDX], bf16) for d in DIRS}
    weT_d = {d: inp(f"we_T_{d}", [128, NIDX], bf16) for d in DIRS}
    # x-side combined weight (lhsT): chunks i,i,o,o,a,a,g,g
    xw_d = {d: inp(f"xw_{d}", [E, 8, 128], bf16) for d in DIRS}
    wwih_d = {d: inp(f"wwih_{d}", [E, 6, 128], bf16) for d in DIRS}
    # h-side weight tiles: [128 K-rows, kc, m, 128]
    whh_d = {d: inp(f"whh_{d}", [128, 2, 6, 128], bf16) for d in DIRS}
    wwhh_d = {d: inp(f"wwhh_{d}", [128, 2, 6, 128], bf16) for d in DIRS}
    awhh_d = {d: inp(f"awhh_{d}", [128, 2, 2, 128], bf16) for d in DIRS}
    # bias lhsT rows match pg/pw chunk order
    bg_d = {d: inp(f"biasg_{d}", [8, 128], bf16) for d in DIRS}
    bw_d = {d: inp(f"biasw_{d}", [6, 128], bf16) for d in DIRS}
    selg_d = inp("selg", [8, 8 * LD], bf16)   # sel[k, c*LD+l] = (c==k)
    selw_d = inp("selw", [6, 6 * LD], bf16)
    maskm_d = {d: inp(f"mask_m_{d}", [T, 2 * LD], u8) for d in DIRS}
    maskw_d = {d: inp(f"mask_w_{d}", [T, 2 * LD], u8) for d in DIRS}
    tagw_d = inp("tagw", [128, 2, 2, 32], bf16)   # [K-row, dir, kc, label]

    out_d = nc.declare_dram_parameter("out_tags", [2, 32, T * LD], f32,
                                      isOutput=True)

    with tile.TileContext(nc) as tc:
        with (
            tc.tile_pool(name="const", bufs=1) as cpool,
            tc.tile_pool(name="state", bufs=1) as spool,
            tc.tile_pool(name="work", bufs=2) as wpool,
            tc.tile_pool(name="outp", bufs=4) as opool,
            tc.tile_pool(name="psumGf", bufs=2, space="PSUM") as psGf,
            tc.tile_pool(name="psumGb", bufs=2, space="PSUM") as psGb,
            tc.tile_pool(name="psumWf", bufs=1, space="PSUM") as psWf,
            tc.tile_pool(name="psumWb", bufs=1, space="PSUM") as psWb,
            tc.tile_pool(name="psumT", bufs=2, space="PSUM") as psT,
        ):
            psG = {"f": psGf, "b": psGb}
            psW = {"f": psWf, "b": psWb}

            # ---- load constants ----
            def load(dram, shape, dtype, tag):
                t_ = cpool.tile(list(shape), dtype, tag=tag, name=tag)
                nc.sync.dma_start(out=t_[...], in_=dram[...])
                return t_

            xw = {d: load(xw_d[d], [E, 8, 128], bf16, f"xw{d}") for d in DIRS}
            wwih = {d: load(wwih_d[d], [E, 6, 128], bf16, f"wwih{d}") for d in DIRS}
            whh = {d: load(whh_d[d], [128, 2, 6, 128], bf16, f"whh{d}") for d in DIRS}
            wwhh = {d: load(wwhh_d[d], [128, 2, 6, 128], bf16, f"wwhh{d}")
                    for d in DIRS}
            awhh = {d: load(awhh_d[d], [128, 2, 2, 128], bf16, f"awhh{d}")
                    for d in DIRS}
            bg = {d: load(bg_d[d], [8, 128], bf16, f"bg{d}") for d in DIRS}
            bw_ = {d: load(bw_d[d], [6, 128], bf16, f"bw{d}") for d in DIRS}
            selg = load(selg_d, [8, 8 * LD], bf16, "selg")
            selw = load(selw_d, [6, 6 * LD], bf16, "selw")
            tagw = load(tagw_d, [128, 2, 2, 32], bf16, "tagw")
            x_T = {d: load(xT_d[d], [128, NIDX], bf16, f"xT{d}") for d in DIRS}
            we_T = {d: load(weT_d[d], [128, NIDX], bf16, f"weT{d}") for d in DIRS}

            # masks broadcast to all 128 partitions
            maskm, maskw = {}, {}
            for d in DIRS:
                maskm[d] = cpool.tile([128, T, 2 * LD], u8, tag=f"maskm{d}",
                                      name=f"maskm{d}")
                maskw[d] = cpool.tile([128, T, 2 * LD], u8, tag=f"maskw{d}",
                                      name=f"maskw{d}")
                for md, mt in ((maskm_d[d], maskm[d]), (maskw_d[d], maskw[d])):
                    src = md[...]
                    bsrc = bass.AP(tensor=src.tensor, offset=src.offset,
                                   ap=[[0, 128]] + list(src.ap))
                    nc.sync.dma_start(out=mt[...], in_=bsrc)

            # absorb the mask-DMA completion wait on DVE's vector clock here:
            # copy_predicated (3-AP ISA struct) has only ONE sync-wait slot.
            mwarm = cpool.tile([128, 2 * LD], u8, tag="mwarm", name="mwarm")
            for d in DIRS:
                nc.vector.tensor_copy(mwarm[...], maskm[d][:, 0, :])
                nc.vector.tensor_copy(mwarm[...], maskw[d][:, 0, :])

            # ---- states ----
            h_hist, c_st, pc_st, pc_bf = {}, {}, {}, {}
            for d in DIRS:
                h_hist[d] = spool.tile([128, T + 1, 2, LD], bf16,
                                       tag=f"hh{d}", name=f"hh{d}")
                c_st[d] = spool.tile([128, 2, LD], f32, tag=f"c{d}", name=f"c{d}")
                pc_st[d] = spool.tile([128, 2, LD], f32, tag=f"pc{d}",
                                      name=f"pc{d}")
                pc_bf[d] = spool.tile([128, 2, LD], bf16, tag=f"pcb{d}",
                                      name=f"pcb{d}")
                nc.vector.memset(h_hist[d][:, 0, :, :], 0.0)
                nc.vector.memset(c_st[d][...], 0.0)
                nc.vector.memset(pc_st[d][...], 0.0)
                nc.vector.memset(pc_bf[d][...], 0.0)

            def xcol(tile_, t):
                return tile_[:, t * LD: (t + 1) * LD]

            def emit_pg_inject(d, pg, t):
                """bias + x-side products for step t into pg (opens group)."""
                nc.tensor.matmul(pg[...], bg[d][...], selg[...],
                                 start=True, stop=False)
                for m in range(8):
                    nc.tensor.matmul(pg[:, m:m + 1, :], xw[d][:, m, :],
                                     xcol(x_T[d], t), start=False, stop=False)

            def emit_pg_h(d, pg, t):
                """pre-h into pg for step t (reads h_{t-1} = slot t)."""
                for kc in range(2):
                    for mi in range(6):
                        nc.tensor.matmul(pg[:, PG_HCH[mi]:PG_HCH[mi] + 1, :],
                                         whh[d][:, kc, mi, :],
                                         h_hist[d][:, t, kc, :],
                                         start=False, stop=False)

            def emit_pg_alpha(d, pg):
                """alpha-h (pc) into pg; closes the group."""
                n = 0
                for kc in range(2):
                    for m in range(2):
                        n += 1
                        nc.tensor.matmul(pg[:, 4 + m:5 + m, :],
                                         awhh[d][:, kc, m, :],
                                         pc_bf[d][:, kc, :],
                                         start=False, stop=(n == 4))

            def emit_pw(d, pw, t):
                nc.tensor.matmul(pw[...], bw_[d][...], selw[...],
                                 start=True, stop=False)
                for m in range(6):
                    nc.tensor.matmul(pw[:, m:m + 1, :], wwih[d][:, m, :],
                                     xcol(we_T[d], t), start=False, stop=False)

            def emit_pw_h(d, pw, t):
                n = 0
                for kc in range(2):
                    for m in range(6):
                        n += 1
                        nc.tensor.matmul(pw[:, m:m + 1, :],
                                         wwhh[d][:, kc, m, :],
                                         h_hist[d][:, t + 1, kc, :],
                                         start=False, stop=(n == 12))

            def wt(shape, tg):
                return wpool.tile(shape, f32, tag=tg, name=tg)

            # prologue: pg for step 0 (h_{-1}=0, pc=0 tiles)
            pg = {}
            for d in DIRS:
                pg[d] = psG[d].tile([128, 8, LD], f32, tag=f"pg{d}",
                                    name=f"pg{d}")
                emit_pg_inject(d, pg[d], 0)
                emit_pg_h(d, pg[d], 0)
                emit_pg_alpha(d, pg[d])

            for t in range(T):
                ga, gt_, spre, s_ = {}, {}, {}, {}
                # V1 activations: i,o,alpha in one sigmoid; g tanh
                for d in DIRS:
                    ga[d] = wt([128, 6, LD], f"ga{d}")
                    nc.scalar.activation(ga[d][...], pg[d][:, 0:6, :], Sig)
                    gt_[d] = wt([128, 2, LD], f"gt{d}")
                    nc.scalar.activation(gt_[d][...], pg[d][:, 6:8, :], Tanh)
                for d in DIRS:
                    spre[d] = wt([128, 2, LD], f"spre{d}")
                    nc.vector.tensor_sub(spre[d][...], ga[d][:, 0:2, :],
                                         ga[d][:, 4:6, :])
                # word-cell inject can start early (only x-dependent)
                pw = {}
                for d in DIRS:
                    pw[d] = psW[d].tile([128, 6, LD], f32, tag=f"pw{d}",
                                        name=f"pw{d}")
                    emit_pw(d, pw[d], t)
                for d in DIRS:
                    s_[d] = wt([128, 2, LD], f"s{d}")
                    nc.scalar.activation(s_[d][...], spre[d][...], Sig)
                # c update: q/e via in-place predicated copies
                for d in DIRS:
                    m2 = maskm[d][:, t, :]
                    nc.vector.copy_predicated(_flat2(ga[d][:, 0:2, :]), m2,
                                              _flat2(s_[d][...]))
                    nc.vector.copy_predicated(_flat2(c_st[d][...]), m2,
                                              _flat2(pc_st[d][...]))
                    d2 = wt([128, 2, LD], f"d2{d}")
                    nc.gpsimd.tensor_sub(d2[...], gt_[d][...], c_st[d][...])
                    ed = wt([128, 2, LD], f"ed{d}")
                    nc.vector.tensor_mul(ed[...], ga[d][:, 0:2, :], d2[...])
                    nc.vector.tensor_add(c_st[d][...], c_st[d][...], ed[...])
                # next step's bias+x inject into the other pg bank
                pg2 = {}
                if t < T - 1:
                    for d in DIRS:
                        pg2[d] = psG[d].tile([128, 8, LD], f32, tag=f"pg{d}",
                                             name=f"pg{d}")
                        emit_pg_inject(d, pg2[d], t + 1)
                for d in DIRS:
                    tc_ = wt([128, 2, LD], f"tc{d}")
                    nc.scalar.activation(tc_[...], c_st[d][...], Tanh)
                    nc.vector.tensor_mul(h_hist[d][:, t + 1, :, :],
                                         ga[d][:, 2:4, :], tc_[...])
                for d in DIRS:
                    emit_pw_h(d, pw[d], t)
                    if t < T - 1:
                        emit_pg_h(d, pg2[d], t + 1)
                # V2: word cell -> pc
                gv, gwt = {}, {}
                for d in DIRS:
                    gv[d] = wt([128, 4, LD], f"gv{d}")
                    nc.scalar.activation(gv[d][...], pw[d][:, 0:4, :], Sig)
                    gwt[d] = wt([128, 2, LD], f"gwt{d}")
                    nc.scalar.activation(gwt[d][...], pw[d][:, 4:6, :], Tanh)
                for d in DIRS:
                    t2 = wt([128, 2, LD], f"t2{d}")
                    nc.gpsimd.tensor_mul(t2[...], gv[d][:, 0:2, :], gwt[d][...])
                    t1 = wt([128, 2, LD], f"t1{d}")
                    nc.gpsimd.tensor_mul(t1[...], gv[d][:, 2:4, :], c_st[d][...])
                    cw = wt([128, 2, LD], f"cw{d}")
                    nc.vector.tensor_add(cw[...], t1[...], t2[...])
                    w2 = maskw[d][:, t, :]
                    nc.vector.copy_predicated(_flat2(pc_st[d][...]), w2,
                                              _flat2(cw[...]))
                    nc.vector.tensor_copy(pc_bf[d][...], pc_st[d][...])
                if t < T - 1:
                    for d in DIRS:
                        emit_pg_alpha(d, pg2[d])
                    pg = pg2

            # ---- tag projection: out[d, label, tau*LD+lane] ----
            for di, d in enumerate(DIRS):
                for n in range(T // NT):
                    pt = psT.tile([32, NT, LD], f32, tag="pt", name="pt")
                    for kc in range(2):
                        nc.tensor.matmul(
                            pt[...], tagw[:, di, kc, :],
                            h_hist[d][:, 1 + n * NT: 1 + (n + 1) * NT, kc, :],
                            start=(kc == 0), stop=(kc == 1))
                    ob = opool.tile([32, NT, LD], f32, tag="ob", name="ob")
                    nc.vector.tensor_copy(ob[...], pt[...])
                    nc.sync.dma_start(
                        out=out_d[di, :, n * NT * LD: (n + 1) * NT * LD],
                        in_=ob[...])

    nc.compile()
    return nc


# ------------------------- host side -------------------------

def _window_start(w):
    return 0 if w == 0 else C * w - WARM


def _window_t0(w):
    return 0 if w == 0 else WARM


def _masks_for_window(wlen_win):
    """wlen_win [32, T] int -> merge mask m [T,32], has-word hw [T,32] f32,
    replicating the truncated-from-zero pcnt/pvalid recurrence."""
    n = wlen_win.shape[0]
    pcnt = np.full((n,), -1, np.int64)
    pvalid = np.zeros((n,), bool)
    m = np.zeros((T, n), np.float32)
    hw = np.zeros((T, n), np.float32)
    for t in range(T):
        mg = pvalid & (pcnt == 0)
        m[t] = mg.astype(np.float32)
        pvalid = pvalid & ~mg
        pcnt = pcnt - 1
        w = wlen_win[:, t] >= 2
        hw[t] = w.astype(np.float32)
        pcnt = np.where(w, wlen_win[:, t] - 1, pcnt)
        pvalid = pvalid | w
    return m, hw


def _weight_tiles(Wx, Whh, aWx, aWhh, wWx, wWhh, b, ab, wb):
    r = {}
    w6 = Wx.reshape(E, 6, 128)
    aw2 = aWx.reshape(E, 2, 128)
    r["xw"] = np.ascontiguousarray(
        np.concatenate([w6[:, 0:4], aw2, w6[:, 4:6]], axis=1)).astype(bf)
    r["wwih"] = np.ascontiguousarray(wWx.reshape(E, 6, 128)).astype(bf)
    r["whh"] = np.ascontiguousarray(
        Whh.reshape(2, 128, 6, 128).transpose(1, 0, 2, 3)).astype(bf)
    r["wwhh"] = np.ascontiguousarray(
        wWhh.reshape(2, 128, 6, 128).transpose(1, 0, 2, 3)).astype(bf)
    r["awhh"] = np.ascontiguousarray(
        aWhh.reshape(2, 128, 2, 128).transpose(1, 0, 2, 3)).astype(bf)
    b6 = b.reshape(6, 128)
    bgv = np.zeros((8, 128), np.float32)
    bgv[0:4] = b6[0:4]
    bgv[4:6] = ab.reshape(2, 128)
    bgv[6:8] = b6[4:6]
    r["biasg"] = bgv.astype(bf)
    r["biasw"] = wb.reshape(6, 128).astype(bf)
    return r


def _prep(inputs):
    inputs = {k: np.asarray(v) for k, v in inputs.items()}
    cids = inputs["component_ids"].astype(np.int64)
    skip = inputs["skip_input"].astype(np.int64)
    wid, wlen = skip[..., 0], skip[..., 1]

    # reference's skip reversal
    tt = np.arange(S)[None, :]
    valid = wlen > 0
    rev_pos = np.where(valid, S - tt - wlen, S)
    skip_rev = np.zeros((B, S + 1, 2), np.int64)
    bidx = np.broadcast_to(np.arange(B)[:, None], (B, S))
    skip_rev[bidx, rev_pos] = skip * valid[..., None]
    skip_rev = skip_rev[:, :S]
    cids_r = cids[:, ::-1]
    wid_r, wlen_r = skip_rev[..., 0], skip_rev[..., 1]

    emb_bf = inputs["emb"].astype(bf)
    emb_bf32 = emb_bf.astype(np.float32)   # for exact-cast transposed gathers

    wt = {}
    for d, pre in (("f", "fw_"), ("b", "bw_")):
        a = [inputs[pre + n] for n in
             ["Wih", "Whh", "aWih", "aWhh", "wWih", "wWhh", "b", "ab", "wb"]]
        wt[d] = _weight_tiles(*a)

    selg = np.zeros((8, 8 * LD), np.float32)
    for c in range(8):
        selg[c, c * LD:(c + 1) * LD] = 1.0
    selw = np.zeros((6, 6 * LD), np.float32)
    for c in range(6):
        selw[c, c * LD:(c + 1) * LD] = 1.0

    tag = np.zeros((128, 2, 2, 32), np.float32)
    tw = inputs["tag_W"]          # [512, 32]
    for di in range(2):
        for kc in range(2):
            tag[:, di, kc, :] = tw[256 * di + 128 * kc: 256 * di + 128 * (kc + 1), :]

    shared = {"selg": selg.astype(bf), "selw": selw.astype(bf),
              "tagw": tag.astype(bf)}
    for d in DIRS:
        for nm in ["xw", "wwih", "whh", "wwhh", "awhh", "biasg", "biasw"]:
            shared[f"{nm}_{d}"] = wt[d][nm]

    src = {"f": (cids, wid, wlen), "b": (cids_r, wid_r, wlen_r)}

    in_maps = []
    for k in range(NCORES):
        im = dict(shared)
        for d in DIRS:
            cd, wd, ld = src[d]
            xb, wb2, mm, mw = [], [], [], []
            for j in range(G):
                st = _window_start(G * k + j)
                xb.append(cd[:, st:st + T].T)          # [T, 32]
                wb2.append(wd[:, st:st + T].T)
                m_, h_ = _masks_for_window(ld[:, st:st + T])
                mm.append(m_)
                mw.append(h_)
            xflat = np.concatenate(xb, axis=1).reshape(-1)    # [T*LD]
            wflat = np.concatenate(wb2, axis=1).reshape(-1)
            im[f"x_T_{d}"] = np.ascontiguousarray(
                emb_bf32[xflat, :].T).astype(bf)
            im[f"we_T_{d}"] = np.ascontiguousarray(
                emb_bf32[wflat, :].T).astype(bf)
            mmc = np.concatenate(mm, axis=1).astype(np.uint8)
            mwc = np.concatenate(mw, axis=1).astype(np.uint8)
            im[f"mask_m_{d}"] = np.repeat(mmc[:, None, :], 2,
                                          axis=1).reshape(T, 2 * LD)
            im[f"mask_w_{d}"] = np.repeat(mwc[:, None, :], 2,
                                          axis=1).reshape(T, 2 * LD)
        in_maps.append(im)
    return in_maps


def _postprocess(results, inputs):
    tag_b = np.asarray(inputs["tag_b"])
    out = np.zeros((B, S, L), np.float32)
    for k in range(NCORES):
        arr = results[k]["out_tags"]              # [2, 32, T*LD]
        fwp = arr[0].reshape(L, T, G, 32)         # [label, tau, win, batch]
        bwp = arr[1].reshape(L, T, G, 32)
        for j in range(G):
            w = G * k + j
            st = _window_start(w)
            t0 = _window_t0(w)
            gsl = np.arange(C) + st + t0
            out[:, gsl, :] += fwp[:, t0:t0 + C, j, :].transpose(2, 1, 0)
            out[:, S - 1 - gsl, :] += bwp[:, t0:t0 + C, j, :].transpose(2, 1, 0)
    return out + tag_b[None, None, :]


def _ensure_ntff_hook():
    """The image's antenv lacks axon_hooks; shim it so trace=True works."""
    import sys
    import types
    try:
        from antenv.axon_hooks import get_axon_ntff_profile_hook  # noqa: F401
        return
    except ImportError:
        pass
    import antenv
    from trn_agent_boot.trn_boot import _ntff_profile_via_ctypes
    mod = types.ModuleType("antenv.axon_hooks")
    _state = {"h": _ntff_profile_via_ctypes("/opt/axon/libaxon_pjrt.so")}
    mod.set_axon_ntff_profile_hook = lambda h: _state.__setitem__("h", h)
    mod.get_axon_ntff_profile_hook = lambda: _state["h"]
    sys.modules["antenv.axon_hooks"] = mod
    antenv.axon_hooks = mod


def run(inputs, trace=False):
    if trace:
        _ensure_ntff_hook()
    if "nc" not in _CACHE:
        _CACHE["nc"] = _build_bass()
    nc = _CACHE["nc"]
    in_maps = _prep(inputs)
    res = run_bass_kernel_spmd(nc, in_maps, core_ids=list(range(NCORES)),
                               trace=trace)
    out = _postprocess(res.results, {k: np.asarray(v) for k, v in inputs.items()})
    return out, res


def kernel(**inputs):
    out, _ = run(inputs, trace=False)
    return out


# revision 8
# speedup vs baseline: 2.1091x; 1.0406x over previous
"""LatticeLSTM (BiLSTM w/ word cells) Trainium2 kernel, v2.

Sharding: time-sharded, 2 windows per core. The 512-step scan splits
into 16 windows of C=32 owned steps; core k runs windows {2k, 2k+1}
simultaneously as extra lanes. Each window computes T = W + C = 48
local steps, where the first W=16 steps warm up the state from zero
(coupled forget gate contracts ~0.5/step, so warm-start error is
~2^-14 by the first owned step). Window 0 starts at global step 0
with no warm-up (its last W steps are wasted instead). No
collectives.

Device layout: feature/gate index on SBUF partitions, lanes on the
free dim. Lanes per direction L = 64 (2 windows x 32 batch). The fw
and bw recurrences are fully independent streams with separate PSUM
banks, weights, and work tiles, emitted interleaved so their serial
elementwise chains pipeline across the Act/DVE/GpSimd engines.

Per-step PSUM banks (per direction):
  pg [128, 8, 64]: pre-gates, chunk order i(2) o(2) alpha(2) g(2) so
     one Sigmoid covers chunks 0:6 and one Tanh covers 6:8.
  pw [128, 6, 64]: word gates iw(2) fw(2) gw(2).
Biases are injected by a [K=chunks, M=128] x [K, chunks*lanes]
selector matmul (start=True clears the bank), then x-projections and
h-matmuls accumulate on top.

The three mask-lerps (merge cell, merge input gate, pending-word
update) use in-place copy_predicated instead of 3-op lerps. Masks
depend only on the integer word-lattice inputs and are precomputed on
host, as is the skip_input reversal; embedding rows are gathered on
host (bf16).
"""

import numpy as np
import ml_dtypes

import concourse.bass as bass
import concourse.bacc as bacc
import concourse.tile as tile
from concourse import mybir
from concourse.bass_utils import run_bass_kernel_spmd

B, S, E, H, V, L = 32, 512, 128, 256, 21128, 32
NCORES = 8
G = 2                       # windows per core
C = 32                      # owned steps per window
WARM = 16
T = C + WARM                # 48 local steps
LD = 32 * G                 # lanes per direction (64)
NIDX = T * LD               # gathered rows per table per direction
NT = 8                      # tag matmul: steps per N-chunk

f32 = mybir.dt.float32
bf16 = mybir.dt.bfloat16
u8 = mybir.dt.uint8
f16 = mybir.dt.float16
i16 = mybir.dt.int16
Sig = mybir.ActivationFunctionType.Sigmoid
Tanh = mybir.ActivationFunctionType.Tanh

bf = ml_dtypes.bfloat16

_CACHE = {}

DIRS = ("f", "b")
# whh tile chunk mi -> pg chunk (i,i,o,o,g,g -> 0,1,2,3,6,7)
PG_HCH = (0, 1, 2, 3, 6, 7)


def _flat2(ap3):
    """[128, a, b] AP with contiguous free dims -> [128, a*b]."""
    p, (sa, ca), (sb, cb) = ap3.ap
    assert sa == sb * cb, f"non-contiguous free dims: {ap3.ap}"
    return bass.AP(tensor=ap3.tensor, offset=ap3.offset,
                   ap=[p, [sb, ca * cb]])


def _build_bass():
    nc = bacc.Bacc(None, target_bir_lowering=False)

    def inp(name, shape, dtype):
        return nc.declare_dram_parameter(name, list(shape), dtype, isOutput=False)

    xT_d = {d: inp(f"x_T_{d}", [128, NIDX], bf16) for d in DIRS}
    weT_d = {d: inp(f"we_T_{d}", [128, NIDX], bf16) for d in DIRS}
    # x-side combined weight (lhsT): chunks i,i,o,o,a,a,g,g
    xw_d = {d: inp(f"xw_{d}", [E, 8, 128], bf16) for d in DIRS}
    wwih_d = {d: inp(f"wwih_{d}", [E, 6, 128], bf16) for d in DIRS}
    # h-side weight tiles: [128 K-rows, kc, m, 128]
    whh_d = {d: inp(f"whh_{d}", [128, 2, 6, 128], bf16) for d in DIRS}
    wwhh_d = {d: inp(f"wwhh_{d}", [128, 2, 6, 128], bf16) for d in DIRS}
    awhh_d = {d: inp(f"awhh_{d}", [128, 2, 2, 128], bf16) for d in DIRS}
    # bias lhsT rows match pg/pw chunk order
    bg_d = {d: inp(f"biasg_{d}", [8, 128], bf16) for d in DIRS}
    bw_d = {d: inp(f"biasw_{d}", [6, 128], bf16) for d in DIRS}
    selg_d = inp("selg", [8, 8 * LD], bf16)   # sel[k, c*LD+l] = (c==k)
    selw_d = inp("selw", [6, 6 * LD], bf16)
    maskm_d = {d: inp(f"mask_m_{d}", [T, 2 * LD], i16) for d in DIRS}
    maskw_d = {d: inp(f"mask_w_{d}", [T, 2 * LD], i16) for d in DIRS}
    tagw_d = inp("tagw", [128, 2, 2, 32], bf16)   # [K-row, dir, kc, label]

    out_d = nc.declare_dram_parameter("out_tags", [2, 32, T * LD], f32,
                                      isOutput=True)

    with tile.TileContext(nc) as tc:
        with (
            tc.tile_pool(name="const", bufs=1) as cpool,
            tc.tile_pool(name="state", bufs=1) as spool,
            tc.tile_pool(name="work", bufs=2) as wpool,
            tc.tile_pool(name="outp", bufs=4) as opool,
            tc.tile_pool(name="psumGf", bufs=2, space="PSUM") as psGf,
            tc.tile_pool(name="psumGb", bufs=2, space="PSUM") as psGb,
            tc.tile_pool(name="psumWf", bufs=1, space="PSUM") as psWf,
            tc.tile_pool(name="psumWb", bufs=1, space="PSUM") as psWb,
            tc.tile_pool(name="psumT", bufs=2, space="PSUM") as psT,
        ):
            psG = {"f": psGf, "b": psGb}
            psW = {"f": psWf, "b": psWb}

            # ---- load constants ----
            def load(dram, shape, dtype, tag):
                t_ = cpool.tile(list(shape), dtype, tag=tag, name=tag)
                nc.sync.dma_start(out=t_[...], in_=dram[...])
                return t_

            xw = {d: load(xw_d[d], [E, 8, 128], bf16, f"xw{d}") for d in DIRS}
            wwih = {d: load(wwih_d[d], [E, 6, 128], bf16, f"wwih{d}") for d in DIRS}
            whh = {d: load(whh_d[d], [128, 2, 6, 128], bf16, f"whh{d}") for d in DIRS}
            wwhh = {d: load(wwhh_d[d], [128, 2, 6, 128], bf16, f"wwhh{d}")
                    for d in DIRS}
            awhh = {d: load(awhh_d[d], [128, 2, 2, 128], bf16, f"awhh{d}")
                    for d in DIRS}
            bg = {d: load(bg_d[d], [8, 128], bf16, f"bg{d}") for d in DIRS}
            bw_ = {d: load(bw_d[d], [6, 128], bf16, f"bw{d}") for d in DIRS}
            selg = load(selg_d, [8, 8 * LD], bf16, "selg")
            selw = load(selw_d, [6, 6 * LD], bf16, "selw")
            tagw = load(tagw_d, [128, 2, 2, 32], bf16, "tagw")
            x_T = {d: load(xT_d[d], [128, NIDX], bf16, f"xT{d}") for d in DIRS}
            we_T = {d: load(weT_d[d], [128, NIDX], bf16, f"weT{d}") for d in DIRS}

            # masks broadcast to all 128 partitions
            maskm, maskw = {}, {}
            for d in DIRS:
                maskm[d] = cpool.tile([128, T, 2 * LD], i16, tag=f"maskm{d}",
                                      name=f"maskm{d}")
                maskw[d] = cpool.tile([128, T, 2 * LD], i16, tag=f"maskw{d}",
                                      name=f"maskw{d}")
                for md, mt in ((maskm_d[d], maskm[d]), (maskw_d[d], maskw[d])):
                    src = md[...]
                    bsrc = bass.AP(tensor=src.tensor, offset=src.offset,
                                   ap=[[0, 128]] + list(src.ap))
                    nc.sync.dma_start(out=mt[...], in_=bsrc)

            # absorb the mask-DMA completion wait on DVE's vector clock here:
            # copy_predicated (3-AP ISA struct) has only ONE sync-wait slot.
            mwarm = cpool.tile([128, 2 * LD], i16, tag="mwarm", name="mwarm")
            for d in DIRS:
                nc.vector.tensor_copy(mwarm[...], maskm[d][:, 0, :])
                nc.vector.tensor_copy(mwarm[...], maskw[d][:, 0, :])

            # ---- states ----
            h_hist, c_st, pc_st, pc_bf = {}, {}, {}, {}
            for d in DIRS:
                h_hist[d] = spool.tile([128, T + 1, 2, LD], bf16,
                                       tag=f"hh{d}", name=f"hh{d}")
                c_st[d] = spool.tile([128, 2, LD], f16, tag=f"c{d}", name=f"c{d}")
                pc_st[d] = spool.tile([128, 2, LD], f16, tag=f"pc{d}",
                                      name=f"pc{d}")
                pc_bf[d] = spool.tile([128, 2, LD], bf16, tag=f"pcb{d}",
                                      name=f"pcb{d}")
                nc.vector.memset(h_hist[d][:, 0, :, :], 0.0)
                nc.vector.memset(c_st[d][...], 0.0)
                nc.vector.memset(pc_st[d][...], 0.0)
                nc.vector.memset(pc_bf[d][...], 0.0)

            def xcol(tile_, t):
                return tile_[:, t * LD: (t + 1) * LD]

            def emit_pg_inject(d, pg, t):
                """bias + x-side products for step t into pg (opens group)."""
                nc.tensor.matmul(pg[...], bg[d][...], selg[...],
                                 start=True, stop=False)
                for m in range(8):
                    nc.tensor.matmul(pg[:, m:m + 1, :], xw[d][:, m, :],
                                     xcol(x_T[d], t), start=False, stop=False)

            def emit_pg_h(d, pg, t):
                """pre-h into pg for step t (reads h_{t-1} = slot t)."""
                for kc in range(2):
                    for mi in range(6):
                        nc.tensor.matmul(pg[:, PG_HCH[mi]:PG_HCH[mi] + 1, :],
                                         whh[d][:, kc, mi, :],
                                         h_hist[d][:, t, kc, :],
                                         start=False, stop=False)

            def emit_pg_alpha(d, pg):
                """alpha-h (pc) into pg; closes the group."""
                n = 0
                for kc in range(2):
                    for m in range(2):
                        n += 1
                        nc.tensor.matmul(pg[:, 4 + m:5 + m, :],
                                         awhh[d][:, kc, m, :],
                                         pc_bf[d][:, kc, :],
                                         start=False, stop=(n == 4))

            def emit_pw(d, pw, t):
                nc.tensor.matmul(pw[...], bw_[d][...], selw[...],
                                 start=True, stop=False)
                for m in range(6):
                    nc.tensor.matmul(pw[:, m:m + 1, :], wwih[d][:, m, :],
                                     xcol(we_T[d], t), start=False, stop=False)

            def emit_pw_h(d, pw, t):
                n = 0
                for kc in range(2):
                    for m in range(6):
                        n += 1
                        nc.tensor.matmul(pw[:, m:m + 1, :],
                                         wwhh[d][:, kc, m, :],
                                         h_hist[d][:, t + 1, kc, :],
                                         start=False, stop=(n == 12))

            def wt(shape, tg):
                return wpool.tile(shape, f16, tag=tg, name=tg)

            # prologue: pg for step 0 (h_{-1}=0, pc=0 tiles)
            pg, pw = {}, {}
            for d in DIRS:
                pg[d] = psG[d].tile([128, 8, LD], f32, tag=f"pg{d}",
                                    name=f"pg{d}")
                emit_pg_inject(d, pg[d], 0)
                emit_pg_h(d, pg[d], 0)
                emit_pg_alpha(d, pg[d])

            def h1(d, t):
                """V1: gates -> c_new, h_t; plus pw inject + pw_h matmuls."""
                pw[d] = psW[d].tile([128, 6, LD], f32, tag=f"pw{d}",
                                    name=f"pw{d}")
                emit_pw(d, pw[d], t)
                ga = wt([128, 6, LD], f"ga{d}")
                nc.scalar.activation(ga[...], pg[d][:, 0:6, :], Sig)
                gt_ = wt([128, 2, LD], f"gt{d}")
                nc.scalar.activation(gt_[...], pg[d][:, 6:8, :], Tanh)
                spre = wt([128, 2, LD], f"spre{d}")
                nc.vector.tensor_sub(spre[...], ga[:, 0:2, :], ga[:, 4:6, :])
                s_ = wt([128, 2, LD], f"s{d}")
                nc.scalar.activation(s_[...], spre[...], Sig)
                m2 = maskm[d][:, t, :]
                nc.vector.copy_predicated(_flat2(ga[:, 0:2, :]), m2,
                                          _flat2(s_[...]))
                nc.vector.copy_predicated(_flat2(c_st[d][...]), m2,
                                          _flat2(pc_st[d][...]))
                d2 = wt([128, 2, LD], f"d2{d}")
                nc.gpsimd.tensor_sub(d2[...], gt_[...], c_st[d][...])
                ed = wt([128, 2, LD], f"ed{d}")
                nc.vector.tensor_mul(ed[...], ga[:, 0:2, :], d2[...])
                nc.vector.tensor_add(c_st[d][...], c_st[d][...], ed[...])
                tc_ = wt([128, 2, LD], f"tc{d}")
                nc.scalar.activation(tc_[...], c_st[d][...], Tanh)
                nc.vector.tensor_mul(h_hist[d][:, t + 1, :, :],
                                     ga[:, 2:4, :], tc_[...])
                emit_pw_h(d, pw[d], t)

            def h2(d, t):
                """V2: word cell -> pc; plus next step's pg group."""
                pgn = None
                if t < T - 1:
                    pgn = psG[d].tile([128, 8, LD], f32, tag=f"pg{d}",
                                      name=f"pg{d}")
                    emit_pg_inject(d, pgn, t + 1)
                gv = wt([128, 4, LD], f"gv{d}")
                nc.scalar.activation(gv[...], pw[d][:, 0:4, :], Sig)
                gwt_ = wt([128, 2, LD], f"gwt{d}")
                nc.scalar.activation(gwt_[...], pw[d][:, 4:6, :], Tanh)
                t2 = wt([128, 2, LD], f"t2{d}")
                nc.gpsimd.tensor_mul(t2[...], gv[:, 0:2, :], gwt_[...])
                t1 = wt([128, 2, LD], f"t1{d}")
                nc.gpsimd.tensor_mul(t1[...], gv[:, 2:4, :], c_st[d][...])
                cw = wt([128, 2, LD], f"cw{d}")
                nc.vector.tensor_add(cw[...], t1[...], t2[...])
                w2 = maskw[d][:, t, :]
                nc.vector.copy_predicated(_flat2(pc_st[d][...]), w2,
                                          _flat2(cw[...]))
                nc.vector.tensor_copy(pc_bf[d][...], pc_st[d][...])
                if t < T - 1:
                    emit_pg_h(d, pgn, t + 1)
                    emit_pg_alpha(d, pgn)
                    pg[d] = pgn

            # software-pipeline the two independent streams half a step
            # apart so their serial chains interleave across engines.
            h1("f", 0)
            for t in range(T):
                h2("f", t)
                h1("b", t)
                if t < T - 1:
                    h1("f", t + 1)
                h2("b", t)

            # ---- tag projection: out[d, label, tau*LD+lane] ----
            for di, d in enumerate(DIRS):
                for n in range(T // NT):
                    pt = psT.tile([32, NT, LD], f32, tag="pt", name="pt")
                    for kc in range(2):
                        nc.tensor.matmul(
                            pt[...], tagw[:, di, kc, :],
                            h_hist[d][:, 1 + n * NT: 1 + (n + 1) * NT, kc, :],
                            start=(kc == 0), stop=(kc == 1))
                    ob = opool.tile([32, NT, LD], f32, tag="ob", name="ob")
                    nc.vector.tensor_copy(ob[...], pt[...])
                    nc.sync.dma_start(
                        out=out_d[di, :, n * NT * LD: (n + 1) * NT * LD],
                        in_=ob[...])

    nc.compile()
    return nc


# ------------------------- host side -------------------------

def _window_start(w):
    return 0 if w == 0 else C * w - WARM


def _window_t0(w):
    return 0 if w == 0 else WARM


def _masks_for_window(wlen_win):
    """wlen_win [32, T] int -> merge mask m [T,32], has-word hw [T,32] f32,
    replicating the truncated-from-zero pcnt/pvalid recurrence."""
    n = wlen_win.shape[0]
    pcnt = np.full((n,), -1, np.int64)
    pvalid = np.zeros((n,), bool)
    m = np.zeros((T, n), np.float32)
    hw = np.zeros((T, n), np.float32)
    for t in range(T):
        mg = pvalid & (pcnt == 0)
        m[t] = mg.astype(np.float32)
        pvalid = pvalid & ~mg
        pcnt = pcnt - 1
        w = wlen_win[:, t] >= 2
        hw[t] = w.astype(np.float32)
        pcnt = np.where(w, wlen_win[:, t] - 1, pcnt)
        pvalid = pvalid | w
    return m, hw


def _weight_tiles(Wx, Whh, aWx, aWhh, wWx, wWhh, b, ab, wb):
    r = {}
    w6 = Wx.reshape(E, 6, 128)
    aw2 = aWx.reshape(E, 2, 128)
    r["xw"] = np.ascontiguousarray(
        np.concatenate([w6[:, 0:4], aw2, w6[:, 4:6]], axis=1)).astype(bf)
    r["wwih"] = np.ascontiguousarray(wWx.reshape(E, 6, 128)).astype(bf)
    r["whh"] = np.ascontiguousarray(
        Whh.reshape(2, 128, 6, 128).transpose(1, 0, 2, 3)).astype(bf)
    r["wwhh"] = np.ascontiguousarray(
        wWhh.reshape(2, 128, 6, 128).transpose(1, 0, 2, 3)).astype(bf)
    r["awhh"] = np.ascontiguousarray(
        aWhh.reshape(2, 128, 2, 128).transpose(1, 0, 2, 3)).astype(bf)
    b6 = b.reshape(6, 128)
    bgv = np.zeros((8, 128), np.float32)
    bgv[0:4] = b6[0:4]
    bgv[4:6] = ab.reshape(2, 128)
    bgv[6:8] = b6[4:6]
    r["biasg"] = bgv.astype(bf)
    r["biasw"] = wb.reshape(6, 128).astype(bf)
    return r


def _prep(inputs):
    inputs = {k: np.asarray(v) for k, v in inputs.items()}
    cids = inputs["component_ids"].astype(np.int64)
    skip = inputs["skip_input"].astype(np.int64)
    wid, wlen = skip[..., 0], skip[..., 1]

    # reference's skip reversal
    tt = np.arange(S)[None, :]
    valid = wlen > 0
    rev_pos = np.where(valid, S - tt - wlen, S)
    skip_rev = np.zeros((B, S + 1, 2), np.int64)
    bidx = np.broadcast_to(np.arange(B)[:, None], (B, S))
    skip_rev[bidx, rev_pos] = skip * valid[..., None]
    skip_rev = skip_rev[:, :S]
    cids_r = cids[:, ::-1]
    wid_r, wlen_r = skip_rev[..., 0], skip_rev[..., 1]

    emb_bf = inputs["emb"].astype(bf)
    emb_bf32 = emb_bf.astype(np.float32)   # for exact-cast transposed gathers

    wt = {}
    for d, pre in (("f", "fw_"), ("b", "bw_")):
        a = [inputs[pre + n] for n in
             ["Wih", "Whh", "aWih", "aWhh", "wWih", "wWhh", "b", "ab", "wb"]]
        wt[d] = _weight_tiles(*a)

    selg = np.zeros((8, 8 * LD), np.float32)
    for c in range(8):
        selg[c, c * LD:(c + 1) * LD] = 1.0
    selw = np.zeros((6, 6 * LD), np.float32)
    for c in range(6):
        selw[c, c * LD:(c + 1) * LD] = 1.0

    tag = np.zeros((128, 2, 2, 32), np.float32)
    tw = inputs["tag_W"]          # [512, 32]
    for di in range(2):
        for kc in range(2):
            tag[:, di, kc, :] = tw[256 * di + 128 * kc: 256 * di + 128 * (kc + 1), :]

    shared = {"selg": selg.astype(bf), "selw": selw.astype(bf),
              "tagw": tag.astype(bf)}
    for d in DIRS:
        for nm in ["xw", "wwih", "whh", "wwhh", "awhh", "biasg", "biasw"]:
            shared[f"{nm}_{d}"] = wt[d][nm]

    src = {"f": (cids, wid, wlen), "b": (cids_r, wid_r, wlen_r)}

    in_maps = []
    for k in range(NCORES):
        im = dict(shared)
        for d in DIRS:
            cd, wd, ld = src[d]
            xb, wb2, mm, mw = [], [], [], []
            for j in range(G):
                st = _window_start(G * k + j)
                xb.append(cd[:, st:st + T].T)          # [T, 32]
                wb2.append(wd[:, st:st + T].T)
                m_, h_ = _masks_for_window(ld[:, st:st + T])
                mm.append(m_)
                mw.append(h_)
            xflat = np.concatenate(xb, axis=1).reshape(-1)    # [T*LD]
            wflat = np.concatenate(wb2, axis=1).reshape(-1)
            im[f"x_T_{d}"] = np.ascontiguousarray(
                emb_bf32[xflat, :].T).astype(bf)
            im[f"we_T_{d}"] = np.ascontiguousarray(
                emb_bf32[wflat, :].T).astype(bf)
            mmc = np.concatenate(mm, axis=1).astype(np.int16)
            mwc = np.concatenate(mw, axis=1).astype(np.int16)
            im[f"mask_m_{d}"] = np.repeat(mmc[:, None, :], 2,
                                          axis=1).reshape(T, 2 * LD)
            im[f"mask_w_{d}"] = np.repeat(mwc[:, None, :], 2,
                                          axis=1).reshape(T, 2 * LD)
        in_maps.append(im)
    return in_maps


def _postprocess(results, inputs):
    tag_b = np.asarray(inputs["tag_b"])
    out = np.zeros((B, S, L), np.float32)
    for k in range(NCORES):
        arr = results[k]["out_tags"]              # [2, 32, T*LD]
        fwp = arr[0].reshape(L, T, G, 32)         # [label, tau, win, batch]
        bwp = arr[1].reshape(L, T, G, 32)
        for j in range(G):
            w = G * k + j
            st = _window_start(w)
            t0 = _window_t0(w)
            gsl = np.arange(C) + st + t0
            out[:, gsl, :] += fwp[:, t0:t0 + C, j, :].transpose(2, 1, 0)
            out[:, S - 1 - gsl, :] += bwp[:, t0:t0 + C, j, :].transpose(2, 1, 0)
    return out + tag_b[None, None, :]


def _ensure_ntff_hook():
    """The image's antenv lacks axon_hooks; shim it so trace=True works."""
    import sys
    import types
    try:
        from antenv.axon_hooks import get_axon_ntff_profile_hook  # noqa: F401
        return
    except ImportError:
        pass
    import antenv
    from trn_agent_boot.trn_boot import _ntff_profile_via_ctypes
    mod = types.ModuleType("antenv.axon_hooks")
    _state = {"h": _ntff_profile_via_ctypes("/opt/axon/libaxon_pjrt.so")}
    mod.set_axon_ntff_profile_hook = lambda h: _state.__setitem__("h", h)
    mod.get_axon_ntff_profile_hook = lambda: _state["h"]
    sys.modules["antenv.axon_hooks"] = mod
    antenv.axon_hooks = mod


def run(inputs, trace=False):
    if trace:
        _ensure_ntff_hook()
    if "nc" not in _CACHE:
        _CACHE["nc"] = _build_bass()
    nc = _CACHE["nc"]
    in_maps = _prep(inputs)
    res = run_bass_kernel_spmd(nc, in_maps, core_ids=list(range(NCORES)),
                               trace=trace)
    out = _postprocess(res.results, {k: np.asarray(v) for k, v in inputs.items()})
    return out, res


def kernel(**inputs):
    out, _ = run(inputs, trace=False)
    return out
